# revision 34
# baseline (speedup 1.0000x reference)
"""Trainium2 Bass kernel: MixedScore MultiHeadAttention (v4 hybrid).

Math (per batch b, head h):
  S[r,c]   = (q[b,h,r,:] . k[b,h,c,:]) / 4
  t_m[r,c] = a_m*S + c_m*Q + b1_m          (Q = cost_mat[b])
  mixed    = sum_m w2_m * relu(t_m)  (+ b2, dropped: softmax shift-invariant)
  out      = softmax_c(mixed) @ v

Folding |w2_m| into (a_m, c_m, b1_m) gives  mixed = sum_m s_m * relu(A_m*S + C_m*Q + B_m)
with s_m = sign(w2_m).

v4 strategy (per core; core = (b, half-of-heads), 8 heads/core):
  - qh SBUF (128, 8, 512) fp32r: partitions 0:64 = S^T 64-c chunk j
    (rewritten per head from PSUM), partitions 64:128 = cost^T rows
    (DMA'd once, reused by all heads).
  - S^T: fp32r K=16 matmuls, out (64, 512); two chunks share a
    (64, 2, 512) PSUM tile -> one (64, 1024) copy per 2 chunks.
  - mix1: fp32r SELF-LOADING matmuls (no separate LDWEIGHTS -> ~234ns
    pipelined spacing): stationary (128, 128) block per (head, group g)
    encodes A_m at row 8g+c8 / C_m at row 64+8g+c8, col c8*16+m. Out
    (128 = 8c x 16m, 512 r). Groups (2t, 2t+1) share a 2-bank PSUM pair.
  - relu + per-partition bias B[p%16]: ONE (128, 1024) instr per pair,
    writing fp8 r1 (128, 2, 512) = exactly the mix2 DoubleRow k-tile
    pair. Split ACT/DVE ~5:4 (Pool cannot read PSUM on TRN2).
  - mix2: fp8 DoubleRow, K = 2x128 (r1 slots = groups 2t, 2t+1), lhsT
    (128, 2, 64): group 2t's 16c at cols 16t+c8 / 16t+8+c8; out (64, 512)
    at partition base 64*(j%2) of pmx (128, 512), ACCUMULATED over
    t = 0..4 (rows outside the pair contribute zeros). Base stays 0/64
    (the ISA only allows AP partition bases {0,32,64}). Emitted one chunk
    behind mix1 (software pipeline) so relus are ready.
  - exp on ACT ((128, 512) per chunk-pair; no max subtraction, |logit|
    <= ~21 is fp32-safe), emitted ahead of later relus in the ACT queue.
  - PV: fp32r; lhsT = [v 128-c chunk | ones] (128, 17): col 16
    accumulates the softmax denominator; divide on host. PV matmuls are
    deferred under the NEXT chunk's mix1 block so the PE never stalls on
    exp; the last pair's PV lands inside the next head's first chunk.
fp8 is used only for relu values + mix2 weights (~0.7% rel err vs the
2e-2 tolerance); scores stay fp32 end-to-end. PSUM banks: psS 2 +
p1 2x2 + pmx 1 + pvT 1 = 8.
"""

import os
import sys

import numpy as np

sys.path.insert(0, "/opt/trn_rl_repo")

import ml_dtypes  # noqa: E402

import concourse.bass as bass  # noqa: E402
import concourse.mybir as mybir  # noqa: E402
from concourse import bacc, tile  # noqa: E402
from concourse.bass_utils import run_bass_kernel_spmd  # noqa: E402

FP = mybir.dt.float32
FPR = mybir.dt.float32r
F8 = mybir.dt.float8e4
DR = mybir.MatmulPerfMode.DoubleRow
E4M3 = ml_dtypes.float8_e4m3
B, H, R, C, D, M = 4, 16, 512, 512, 16, 16
HPC = 8  # heads per core
NCORES = 8

AF = mybir.ActivationFunctionType
ALU = mybir.AluOpType

last_results = None  # BassKernelResults of the most recent run (for test.py)


def build_bass():
    nc = bacc.Bacc(None, target_bir_lowering=False, debug=False)

    qT = nc.declare_dram_parameter("qT", [D, HPC, R], FPR, isOutput=False)
    kT = nc.declare_dram_parameter("kT", [D, HPC, C], FPR, isOutput=False)
    costT = nc.declare_dram_parameter("costT", [C, R], FPR, isOutput=False)
    vx = nc.declare_dram_parameter("vx", [128, HPC, 4, 17], FPR, isOutput=False)
    w1s = nc.declare_dram_parameter("w1s", [128, HPC, 8, 128], FPR, isOutput=False)
    w2s = nc.declare_dram_parameter(
        "w2s", [128, HPC, 2, 4, 2, 128], F8, isOutput=False
    )
    bvs = nc.declare_dram_parameter("bvs", [128, HPC], FP, isOutput=False)
    outp = nc.declare_dram_parameter("out", [HPC, D + 1, R], FP, isOutput=True)

    with tile.TileContext(nc) as tc:
        with (
            tc.tile_pool(name="const", bufs=1) as constp,
            tc.tile_pool(name="qhp", bufs=1) as qhp,
            tc.tile_pool(name="r1", bufs=6) as r1p,
            tc.tile_pool(name="wexp", bufs=4) as wexpp,
            tc.tile_pool(name="osb", bufs=4) as osbp,
            tc.tile_pool(name="psS", bufs=1, space="PSUM") as psSp,
            tc.tile_pool(name="ps1", bufs=4, space="PSUM") as ps1p,
            tc.tile_pool(name="psmx", bufs=1, space="PSUM") as psmxp,
            tc.tile_pool(name="pspv", bufs=1, space="PSUM") as pspvp,
        ):
            qT_sb = constp.tile([D, HPC, R], FPR)
            kT_sb = constp.tile([D, HPC, C], FPR)
            vx_sb = constp.tile([128, HPC, 4, 17], FPR)
            w1_sb = constp.tile([128, HPC, 8, 128], FPR)
            w2_sb = constp.tile([128, HPC, 2, 4, 2, 128], F8)
            bv_sb = constp.tile([128, HPC], FP)

            qh = [
                qhp.tile([128, 8, 512], FPR, name=f"qh{i}", tag=f"qh{i}")
                for i in range(2)
            ]
            nc.sync.dma_start(out=qT_sb[:, 0], in_=qT[:, 0])
            nc.sync.dma_start(out=kT_sb[:, 0], in_=kT[:, 0])
            nc.sync.dma_start(out=qT_sb[:, 1:], in_=qT[:, 1:])
            nc.sync.dma_start(out=kT_sb[:, 1:], in_=kT[:, 1:])
            nc.sync.dma_start(out=w1_sb[:, 0], in_=w1s[:, 0])
            nc.sync.dma_start(out=w2_sb[:, 0], in_=w2s[:, 0])
            nc.sync.dma_start(out=bv_sb[:], in_=bvs[:])
            for j in range(8):
                nc.sync.dma_start(
                    out=qh[0][64:128, j, :], in_=costT[64 * j : 64 * j + 64, :]
                )
            for j in range(8):
                nc.sync.dma_start(
                    out=qh[1][64:128, j, :], in_=costT[64 * j : 64 * j + 64, :]
                )
            for hhl in range(1, HPC):
                nc.sync.dma_start(out=w1_sb[:, hhl], in_=w1s[:, hhl])
            nc.sync.dma_start(out=w2_sb[:, 1:], in_=w2s[:, 1:])
            nc.sync.dma_start(out=vx_sb[:], in_=vx[:])

            copy_rr = [0]

            def any_copy(out, in_):
                i = copy_rr[0] % 2
                copy_rr[0] += 1
                if i == 0:
                    nc.scalar.copy(out=out, in_=in_)
                else:
                    nc.vector.tensor_copy(out=out, in_=in_)

            def emit_st_pair(hs, j):
                # S^T chunks (j, j+1), two back-to-back K=16 matmuls into a
                # (64, 2, 512) PSUM tile, one (64, 1024) copy into qh
                qdst = qh[hs % 2]
                ps = psSp.tile([64, 2, 512], FP, name="ps", tag="ps")
                for jj in (j, j + 1):
                    nc.tensor.matmul(
                        ps[:, jj % 2, :],
                        lhsT=kT_sb[:, hs, 64 * jj : 64 * jj + 64],
                        rhs=qT_sb[:, hs, :],
                        start=True,
                        stop=True,
                    )
                any_copy(qdst[0:64, j : j + 2, :], ps[:])

            relu_rr = [0]

            def emit_relu1(r1slot, p1, bias):
                # one (128, 512) relu single, alternating ACT/DVE
                i = relu_rr[0] % 2
                relu_rr[0] += 1
                if i == 0:
                    nc.scalar.activation(r1slot, p1[:], AF.Relu, bias=bias)
                else:
                    nc.vector.tensor_scalar(
                        out=r1slot,
                        in0=p1[:],
                        scalar1=bias,
                        scalar2=0.0,
                        op0=ALU.add,
                        op1=ALU.max,
                    )

            for j in range(0, 8, 2):
                emit_st_pair(0, j)

            deferred = []  # PV/out-copy closures flushed under later mix1

            def flush_deferred():
                while deferred:
                    deferred.pop(0)()

            pmx_cur = [None]

            def emit_mix2(hh, j, r1s, pvT):
                # mix2 for chunk j (called one chunk later); lazily create
                # the pmx tile per chunk-pair; after the odd chunk, exp +
                # deferred PV for the pair
                if j % 2 == 0:
                    pmx_cur[0] = psmxp.tile([128, 512], FP, name="pmx", tag="pmx")
                pmx = pmx_cur[0]
                # out is the FULL 128 partitions (DR rejects col offsets);
                # the stationary's zero columns pad the other chunk's half.
                # One accumulation group spans both chunks of the pair.
                for t in range(4):
                    nc.tensor.matmul(
                        pmx[:, :],
                        lhsT=w2_sb[:, hh, j % 2, t],
                        rhs=r1s[t][:],
                        start=(j % 2 == 0 and t == 0),
                        stop=(j % 2 == 1 and t == 3),
                        perf_mode=DR,
                    )
                if j % 2 == 1:
                    k = j // 2
                    wx = wexpp.tile([128, 512], FPR, name="wx", tag="wx")
                    nc.scalar.activation(wx[:], pmx[:], AF.Exp)

                    def emit_pv(hh=hh, k=k, wx=wx, pvT=pvT):
                        nc.tensor.matmul(
                            pvT[:],
                            lhsT=vx_sb[:, hh, k, :],
                            rhs=wx[:],
                            start=(k == 0),
                            stop=(k == 3),
                        )
                        if k == 3:
                            ot = osbp.tile([17, 512], FP, name="ot", tag="ot")
                            nc.vector.tensor_copy(out=ot[:], in_=pvT[:])
                            nc.sync.dma_start(out=outp[hh], in_=ot[:])

                    deferred.append(emit_pv)

            for hh in range(HPC):
                qcur = qh[hh % 2]
                pvT = pspvp.tile([17, 512], FP, name="pvT", tag="pvT")
                prev = None  # (j, r1s) for the software-pipelined mix2
                for j in range(8):
                    if hh + 1 < HPC and j % 2 == 0:
                        emit_st_pair(hh + 1, j)
                    r1s = []
                    for t in range(4):
                        r1 = r1p.tile([128, 2, 512], F8, name="r1", tag="r1")
                        for i2 in range(2):
                            g = 2 * t + i2
                            p1 = ps1p.tile([128, 512], FP, name="p1", tag="p1")
                            nc.tensor.matmul(
                                p1[:],
                                lhsT=w1_sb[:, hh, g, :],
                                rhs=qcur[:, j, :],
                                start=True,
                                stop=True,
                            )
                            emit_relu1(r1[:, i2, :], p1, bv_sb[:, hh : hh + 1])
                        r1s.append(r1)
                        if t == 1:
                            flush_deferred()
                    if prev is not None:
                        emit_mix2(hh, prev[0], prev[1], pvT)
                    prev = (j, r1s)
                emit_mix2(hh, prev[0], prev[1], pvT)
            flush_deferred()
    nc.finalize()
    return nc


def prepare_in_maps(q, k, v, cost_mat, mix1_weight, mix1_bias, mix2_weight, mix2_bias):
    q = np.asarray(q, np.float32)
    k = np.asarray(k, np.float32)
    v = np.asarray(v, np.float32)
    cost_mat = np.asarray(cost_mat, np.float32)
    mix1_weight = np.asarray(mix1_weight, np.float32)
    mix1_bias = np.asarray(mix1_bias, np.float32)
    mix2_weight = np.asarray(mix2_weight, np.float32)
    mix2_bias = np.asarray(mix2_bias, np.float32)

    p = np.arange(128)
    in_maps = []
    for core in range(NCORES):
        b = core // 2
        h0 = (core % 2) * HPC
        qTa = np.ascontiguousarray(q[b, h0 : h0 + HPC].transpose(2, 0, 1)) * 0.25
        kTa = np.ascontiguousarray(k[b, h0 : h0 + HPC].transpose(2, 0, 1))
        costTa = np.ascontiguousarray(cost_mat[b].T)
        vv = v[b, h0 : h0 + HPC]  # (HPC, C, D)
        vxa = np.empty((128, HPC, 4, 17), np.float32)
        vxa[:, :, :, :16] = vv.reshape(HPC, 4, 128, 16).transpose(2, 0, 1, 3)
        vxa[:, :, :, 16] = 1.0

        w1 = mix1_weight[h0 : h0 + HPC]  # (HPC, 2, M)
        b1 = mix1_bias[h0 : h0 + HPC]  # (HPC, M)
        w2 = mix2_weight[h0 : h0 + HPC, :, 0]  # (HPC, M)
        aw = np.abs(w2)
        sg = np.sign(w2).astype(np.float32)
        A = (w1[:, 0, :] * aw).astype(np.float32)  # (HPC, M)
        Cc = (w1[:, 1, :] * aw).astype(np.float32)
        Bb = (b1 * aw).astype(np.float32)

        # mix1 stationary (128, HPC, 8, 128): row 8g+c8 -> col c8*16+m = A,
        # row 64+8g+c8 -> col c8*16+m = C
        w1sa = np.zeros((128, HPC, 8, 128), np.float32)
        for g in range(8):
            for c8 in range(8):
                cols = slice(c8 * 16, c8 * 16 + 16)
                w1sa[8 * g + c8, :, g, cols] = A
                w1sa[64 + 8 * g + c8, :, g, cols] = Cc

        # mix2 DoubleRow stationary (128, HPC, 2par, 4t, 2tile, 128): row
        # (c8, m); k-tile0 = group 2t -> col 64*par + 16t+c8; k-tile1 =
        # group 2t+1 -> 64*par + 16t+8+c8; other columns zero
        w2sa = np.zeros((128, HPC, 2, 4, 2, 128), np.float32)
        c8v = p // 16
        mv = p % 16
        for par in range(2):
            for t in range(4):
                w2sa[p, :, par, t, 0, 64 * par + 16 * t + c8v] = sg[:, mv].T
                w2sa[p, :, par, t, 1, 64 * par + 16 * t + 8 + c8v] = sg[:, mv].T

        bvsa = np.tile(Bb.T, (8, 1)).astype(np.float32)  # (128, HPC)

        in_maps.append(
            dict(
                qT=qTa,
                kT=kTa,
                costT=costTa,
                vx=vxa,
                w1s=w1sa,
                w2s=w2sa.astype(E4M3),
                bvs=bvsa,
            )
        )
    return in_maps


def assemble(results):
    full = np.empty((B, R, H * D), np.float32)
    for core in range(NCORES):
        b = core // 2
        c0 = (core % 2) * HPC * D
        o = results[core]["out"]  # (HPC, D+1, R); row D is the softmax denom
        o = o[:, :D, :] / o[:, D : D + 1, :]
        full[b, :, c0 : c0 + HPC * D] = o.transpose(2, 0, 1).reshape(R, HPC * D)
    return full


_nc_cache = None


def _install_ntff_hook():
    """The agent image's antenv lacks axon_hooks; recreate it and register
    the ctypes NTFF profiling hook so trace=True yields exec times."""
    import types

    try:
        import antenv

        try:
            import antenv.axon_hooks  # noqa: F401

            return
        except ImportError:
            pass
        mod = types.ModuleType("antenv.axon_hooks")
        mod._hook = None
        mod.set_axon_ntff_profile_hook = lambda h: setattr(mod, "_hook", h)
        mod.get_axon_ntff_profile_hook = lambda: mod._hook
        sys.modules["antenv.axon_hooks"] = mod
        antenv.axon_hooks = mod
        from trn_agent_boot.trn_boot import _ntff_profile_via_ctypes

        mod._hook = _ntff_profile_via_ctypes("/opt/axon/libaxon_pjrt.so")
    except Exception as e:  # profiling is best-effort
        print(f"ntff hook install failed: {e}", file=sys.stderr)


def kernel(**inputs) -> np.ndarray:
    global _nc_cache, last_results
    if _nc_cache is None:
        _nc_cache = build_bass()
    in_maps = prepare_in_maps(**inputs)
    trace = bool(int(os.environ.get("KERNEL_TRACE", "0")))
    if trace:
        _install_ntff_hook()
        import concourse.bass_utils as bu

        bu.upload_artifacts = lambda tmpdir: f"local:{tmpdir}"
    res = run_bass_kernel_spmd(_nc_cache, in_maps, list(range(NCORES)), trace=trace)
    last_results = res
    return assemble(res.results)


# revision 39
# speedup vs baseline: 1.0398x; 1.0398x over previous
"""Trainium2 Bass kernel: MixedScore MultiHeadAttention (v4 hybrid).

Math (per batch b, head h):
  S[r,c]   = (q[b,h,r,:] . k[b,h,c,:]) / 4
  t_m[r,c] = a_m*S + c_m*Q + b1_m          (Q = cost_mat[b])
  mixed    = sum_m w2_m * relu(t_m)  (+ b2, dropped: softmax shift-invariant)
  out      = softmax_c(mixed) @ v

Folding |w2_m| into (a_m, c_m, b1_m) gives  mixed = sum_m s_m * relu(A_m*S + C_m*Q + B_m)
with s_m = sign(w2_m).

v4 strategy (per core; core = (b, half-of-heads), 8 heads/core):
  - qh SBUF (128, 8, 512) fp32r: partitions 0:64 = S^T 64-c chunk j
    (rewritten per head from PSUM), partitions 64:128 = cost^T rows
    (DMA'd once, reused by all heads).
  - S^T: fp32r K=16 matmuls, out (64, 512); two chunks share a
    (64, 2, 512) PSUM tile -> one (64, 1024) copy per 2 chunks.
  - mix1: fp32r SELF-LOADING matmuls (no separate LDWEIGHTS -> ~234ns
    pipelined spacing): stationary (128, 128) block per (head, group g)
    encodes A_m at row 8g+c8 / C_m at row 64+8g+c8, col c8*16+m. Out
    (128 = 8c x 16m, 512 r). Groups (2t, 2t+1) share a 2-bank PSUM pair.
  - relu + per-partition bias B[p%16]: ONE (128, 1024) instr per pair,
    writing fp8 r1 (128, 2, 512) = exactly the mix2 DoubleRow k-tile
    pair. Split ACT/DVE ~5:4 (Pool cannot read PSUM on TRN2).
  - mix2: fp8 DoubleRow, K = 2x128 (r1 slots = groups 2t, 2t+1), lhsT
    (128, 2, 64): group 2t's 16c at cols 16t+c8 / 16t+8+c8; out (64, 512)
    at partition base 64*(j%2) of pmx (128, 512), ACCUMULATED over
    t = 0..4 (rows outside the pair contribute zeros). Base stays 0/64
    (the ISA only allows AP partition bases {0,32,64}). Emitted one chunk
    behind mix1 (software pipeline) so relus are ready.
  - exp on ACT ((128, 512) per chunk-pair; no max subtraction, |logit|
    <= ~21 is fp32-safe), emitted ahead of later relus in the ACT queue.
  - PV: fp32r; lhsT = [v 128-c chunk | ones] (128, 17): col 16
    accumulates the softmax denominator; divide on host. PV matmuls are
    deferred under the NEXT chunk's mix1 block so the PE never stalls on
    exp; the last pair's PV lands inside the next head's first chunk.
fp8 is used only for relu values + mix2 weights (~0.7% rel err vs the
2e-2 tolerance); scores stay fp32 end-to-end. PSUM banks: psS 2 +
p1 2x2 + pmx 1 + pvT 1 = 8.
"""

import os
import sys

import numpy as np

sys.path.insert(0, "/opt/trn_rl_repo")

import ml_dtypes  # noqa: E402

import concourse.bass as bass  # noqa: E402
import concourse.mybir as mybir  # noqa: E402
from concourse import bacc, tile  # noqa: E402
from concourse.bass_utils import run_bass_kernel_spmd  # noqa: E402

FP = mybir.dt.float32
FPR = mybir.dt.float32r
BF = mybir.dt.bfloat16
F8 = mybir.dt.float8e4
DR = mybir.MatmulPerfMode.DoubleRow
E4M3 = ml_dtypes.float8_e4m3
BF16 = ml_dtypes.bfloat16
B, H, R, C, D, M = 4, 16, 512, 512, 16, 16
HPC = 8  # heads per core
NCORES = 8

AF = mybir.ActivationFunctionType
ALU = mybir.AluOpType

last_results = None  # BassKernelResults of the most recent run (for test.py)


def build_bass():
    nc = bacc.Bacc(None, target_bir_lowering=False, debug=False)

    qT = nc.declare_dram_parameter("qT", [D, HPC, R], FPR, isOutput=False)
    kT = nc.declare_dram_parameter("kT", [D, HPC, C], FPR, isOutput=False)
    costT = nc.declare_dram_parameter("costT", [C, R], BF, isOutput=False)
    vx = nc.declare_dram_parameter("vx", [128, HPC, 4, 17], FPR, isOutput=False)
    w1s = nc.declare_dram_parameter("w1s", [128, HPC, 8, 128], BF, isOutput=False)
    w2s = nc.declare_dram_parameter(
        "w2s", [128, HPC, 2, 4, 2, 128], F8, isOutput=False
    )
    bvs = nc.declare_dram_parameter("bvs", [128, HPC], FP, isOutput=False)
    outp = nc.declare_dram_parameter("out", [HPC, D + 1, R], FP, isOutput=True)

    with tile.TileContext(nc) as tc:
        with (
            tc.tile_pool(name="const", bufs=1) as constp,
            tc.tile_pool(name="qhp", bufs=1) as qhp,
            tc.tile_pool(name="r1", bufs=6) as r1p,
            tc.tile_pool(name="wexp", bufs=4) as wexpp,
            tc.tile_pool(name="osb", bufs=4) as osbp,
            tc.tile_pool(name="psS", bufs=1, space="PSUM") as psSp,
            tc.tile_pool(name="ps1", bufs=4, space="PSUM") as ps1p,
            tc.tile_pool(name="psmx", bufs=1, space="PSUM") as psmxp,
            tc.tile_pool(name="pspv", bufs=1, space="PSUM") as pspvp,
        ):
            qT_sb = constp.tile([D, HPC, R], FPR)
            kT_sb = constp.tile([D, HPC, C], FPR)
            vx_sb = constp.tile([128, HPC, 4, 17], FPR)
            w1_sb = constp.tile([128, HPC, 8, 128], BF)
            w2_sb = constp.tile([128, HPC, 2, 4, 2, 128], F8)
            bv_sb = constp.tile([128, HPC], FP)

            qh = [
                qhp.tile([128, 8, 512], BF, name=f"qh{i}", tag=f"qh{i}")
                for i in range(2)
            ]
            # prologue DMAs split across the two HWDGE queues (sync + scalar)
            nc.sync.dma_start(out=qT_sb[:, 0], in_=qT[:, 0])
            nc.sync.dma_start(out=kT_sb[:, 0], in_=kT[:, 0])
            nc.scalar.dma_start(out=w1_sb[:, 0], in_=w1s[:, 0])
            nc.scalar.dma_start(out=w2_sb[:, 0], in_=w2s[:, 0])
            nc.scalar.dma_start(out=bv_sb[:], in_=bvs[:])
            for j in range(8):
                nc.sync.dma_start(
                    out=qh[0][64:128, j, :], in_=costT[64 * j : 64 * j + 64, :]
                )
            nc.scalar.dma_start(out=qT_sb[:, 1:], in_=qT[:, 1:])
            nc.scalar.dma_start(out=kT_sb[:, 1:], in_=kT[:, 1:])
            for j in range(8):
                nc.sync.dma_start(
                    out=qh[1][64:128, j, :], in_=costT[64 * j : 64 * j + 64, :]
                )
            for hhl in range(1, HPC):
                (nc.sync if hhl % 2 else nc.scalar).dma_start(
                    out=w1_sb[:, hhl], in_=w1s[:, hhl]
                )
            nc.scalar.dma_start(out=w2_sb[:, 1:], in_=w2s[:, 1:])
            nc.sync.dma_start(out=vx_sb[:], in_=vx[:])

            copy_rr = [0]

            def any_copy(out, in_):
                i = copy_rr[0] % 2
                copy_rr[0] += 1
                if i == 0:
                    nc.scalar.copy(out=out, in_=in_)
                else:
                    nc.vector.tensor_copy(out=out, in_=in_)

            def emit_st_pair(hs, j):
                # S^T chunks (j, j+1), two back-to-back K=16 matmuls into a
                # (64, 2, 512) PSUM tile, one (64, 1024) copy into qh
                qdst = qh[hs % 2]
                ps = psSp.tile([64, 2, 512], FP, name="ps", tag="ps")
                for jj in (j, j + 1):
                    nc.tensor.matmul(
                        ps[:, jj % 2, :],
                        lhsT=kT_sb[:, hs, 64 * jj : 64 * jj + 64],
                        rhs=qT_sb[:, hs, :],
                        start=True,
                        stop=True,
                    )
                any_copy(qdst[0:64, j : j + 2, :], ps[:])

            relu_rr = [0]

            def emit_relu1(r1slot, p1, bias):
                # one (128, 512) relu single, alternating ACT/DVE
                i = relu_rr[0] % 2
                relu_rr[0] += 1
                if i == 0:
                    nc.scalar.activation(r1slot, p1[:], AF.Relu, bias=bias)
                else:
                    nc.vector.tensor_scalar(
                        out=r1slot,
                        in0=p1[:],
                        scalar1=bias,
                        scalar2=0.0,
                        op0=ALU.add,
                        op1=ALU.max,
                    )

            for j in range(0, 8, 2):
                emit_st_pair(0, j)

            deferred = []  # PV/out-copy closures flushed under later mix1

            def flush_deferred():
                while deferred:
                    deferred.pop(0)()

            pmx_cur = [None]

            def emit_mix2(hh, j, r1s, pvT):
                # mix2 for chunk j (called one chunk later); lazily create
                # the pmx tile per chunk-pair; after the odd chunk, exp +
                # deferred PV for the pair
                if j % 2 == 0:
                    pmx_cur[0] = psmxp.tile([128, 512], FP, name="pmx", tag="pmx")
                pmx = pmx_cur[0]
                # out is the FULL 128 partitions (DR rejects col offsets);
                # the stationary's zero columns pad the other chunk's half.
                # One accumulation group spans both chunks of the pair.
                for t in range(4):
                    nc.tensor.matmul(
                        pmx[:, :],
                        lhsT=w2_sb[:, hh, j % 2, t],
                        rhs=r1s[t][:],
                        start=(j % 2 == 0 and t == 0),
                        stop=(j % 2 == 1 and t == 3),
                        perf_mode=DR,
                    )
                if j % 2 == 1:
                    k = j // 2
                    wx = wexpp.tile([128, 512], FPR, name="wx", tag="wx")
                    nc.scalar.activation(wx[:], pmx[:], AF.Exp)

                    def emit_pv(hh=hh, k=k, wx=wx, pvT=pvT):
                        nc.tensor.matmul(
                            pvT[:],
                            lhsT=vx_sb[:, hh, k, :],
                            rhs=wx[:],
                            start=(k == 0),
                            stop=(k == 3),
                        )
                        if k == 3:
                            ot = osbp.tile([17, 512], FP, name="ot", tag="ot")
                            nc.vector.tensor_copy(out=ot[:], in_=pvT[:])
                            nc.sync.dma_start(out=outp[hh], in_=ot[:])

                    deferred.append(emit_pv)

            for hh in range(HPC):
                qcur = qh[hh % 2]
                pvT = pspvp.tile([17, 512], FP, name="pvT", tag="pvT")
                prev = None  # (j, r1s) for the software-pipelined mix2
                for j in range(8):
                    if hh + 1 < HPC and j % 2 == 0:
                        emit_st_pair(hh + 1, j)
                    r1s = []
                    for t in range(4):
                        r1 = r1p.tile([128, 2, 512], F8, name="r1", tag="r1")
                        for i2 in range(2):
                            g = 2 * t + i2
                            p1 = ps1p.tile([128, 512], FP, name="p1", tag="p1")
                            nc.tensor.matmul(
                                p1[:],
                                lhsT=w1_sb[:, hh, g, :],
                                rhs=qcur[:, j, :],
                                start=True,
                                stop=True,
                            )
                            emit_relu1(r1[:, i2, :], p1, bv_sb[:, hh : hh + 1])
                        r1s.append(r1)
                        if t == 1:
                            flush_deferred()
                    if prev is not None:
                        emit_mix2(hh, prev[0], prev[1], pvT)
                    prev = (j, r1s)
                emit_mix2(hh, prev[0], prev[1], pvT)
            flush_deferred()
    nc.finalize()
    return nc


def prepare_in_maps(q, k, v, cost_mat, mix1_weight, mix1_bias, mix2_weight, mix2_bias):
    q = np.asarray(q, np.float32)
    k = np.asarray(k, np.float32)
    v = np.asarray(v, np.float32)
    cost_mat = np.asarray(cost_mat, np.float32)
    mix1_weight = np.asarray(mix1_weight, np.float32)
    mix1_bias = np.asarray(mix1_bias, np.float32)
    mix2_weight = np.asarray(mix2_weight, np.float32)
    mix2_bias = np.asarray(mix2_bias, np.float32)

    p = np.arange(128)
    in_maps = []
    for core in range(NCORES):
        b = core // 2
        h0 = (core % 2) * HPC
        qTa = np.ascontiguousarray(q[b, h0 : h0 + HPC].transpose(2, 0, 1)) * 0.25
        kTa = np.ascontiguousarray(k[b, h0 : h0 + HPC].transpose(2, 0, 1))
        costTa = np.ascontiguousarray(cost_mat[b].T)
        vv = v[b, h0 : h0 + HPC]  # (HPC, C, D)
        vxa = np.empty((128, HPC, 4, 17), np.float32)
        vxa[:, :, :, :16] = vv.reshape(HPC, 4, 128, 16).transpose(2, 0, 1, 3)
        vxa[:, :, :, 16] = 1.0

        w1 = mix1_weight[h0 : h0 + HPC]  # (HPC, 2, M)
        b1 = mix1_bias[h0 : h0 + HPC]  # (HPC, M)
        w2 = mix2_weight[h0 : h0 + HPC, :, 0]  # (HPC, M)
        aw = np.abs(w2)
        sg = np.sign(w2).astype(np.float32)
        A = (w1[:, 0, :] * aw).astype(np.float32)  # (HPC, M)
        Cc = (w1[:, 1, :] * aw).astype(np.float32)
        Bb = (b1 * aw).astype(np.float32)

        # mix1 stationary (128, HPC, 8, 128): row 8g+c8 -> col c8*16+m = A,
        # row 64+8g+c8 -> col c8*16+m = C
        w1sa = np.zeros((128, HPC, 8, 128), np.float32)
        for g in range(8):
            for c8 in range(8):
                cols = slice(c8 * 16, c8 * 16 + 16)
                w1sa[8 * g + c8, :, g, cols] = A
                w1sa[64 + 8 * g + c8, :, g, cols] = Cc

        # mix2 DoubleRow stationary (128, HPC, 2par, 4t, 2tile, 128): row
        # (c8, m); k-tile0 = group 2t -> col 64*par + 16t+c8; k-tile1 =
        # group 2t+1 -> 64*par + 16t+8+c8; other columns zero
        w2sa = np.zeros((128, HPC, 2, 4, 2, 128), np.float32)
        c8v = p // 16
        mv = p % 16
        for par in range(2):
            for t in range(4):
                w2sa[p, :, par, t, 0, 64 * par + 16 * t + c8v] = sg[:, mv].T
                w2sa[p, :, par, t, 1, 64 * par + 16 * t + 8 + c8v] = sg[:, mv].T

        bvsa = np.tile(Bb.T, (8, 1)).astype(np.float32)  # (128, HPC)

        in_maps.append(
            dict(
                qT=qTa,
                kT=kTa,
                costT=costTa.astype(BF16),
                vx=vxa,
                w1s=w1sa.astype(BF16),
                w2s=w2sa.astype(E4M3),
                bvs=bvsa,
            )
        )
    return in_maps


def assemble(results):
    full = np.empty((B, R, H * D), np.float32)
    for core in range(NCORES):
        b = core // 2
        c0 = (core % 2) * HPC * D
        o = results[core]["out"]  # (HPC, D+1, R); row D is the softmax denom
        o = o[:, :D, :] / o[:, D : D + 1, :]
        full[b, :, c0 : c0 + HPC * D] = o.transpose(2, 0, 1).reshape(R, HPC * D)
    return full


_nc_cache = None


def _install_ntff_hook():
    """The agent image's antenv lacks axon_hooks; recreate it and register
    the ctypes NTFF profiling hook so trace=True yields exec times."""
    import types

    try:
        import antenv

        try:
            import antenv.axon_hooks  # noqa: F401

            return
        except ImportError:
            pass
        mod = types.ModuleType("antenv.axon_hooks")
        mod._hook = None
        mod.set_axon_ntff_profile_hook = lambda h: setattr(mod, "_hook", h)
        mod.get_axon_ntff_profile_hook = lambda: mod._hook
        sys.modules["antenv.axon_hooks"] = mod
        antenv.axon_hooks = mod
        from trn_agent_boot.trn_boot import _ntff_profile_via_ctypes

        mod._hook = _ntff_profile_via_ctypes("/opt/axon/libaxon_pjrt.so")
    except Exception as e:  # profiling is best-effort
        print(f"ntff hook install failed: {e}", file=sys.stderr)


def kernel(**inputs) -> np.ndarray:
    global _nc_cache, last_results
    if _nc_cache is None:
        _nc_cache = build_bass()
    in_maps = prepare_in_maps(**inputs)
    trace = bool(int(os.environ.get("KERNEL_TRACE", "0")))
    if trace:
        _install_ntff_hook()
        import concourse.bass_utils as bu

        bu.upload_artifacts = lambda tmpdir: f"local:{tmpdir}"
    res = run_bass_kernel_spmd(_nc_cache, in_maps, list(range(NCORES)), trace=trace)
    last_results = res
    return assemble(res.results)
